# revision 22
# baseline (speedup 1.0000x reference)
"""BPR loss kernel for Trainium2 (Bass, raw engine streams), SPMD over 8 cores.

Reference computation (B=32, T=100, N=100000, S=1):
    pos  = output[b, t, labels[b, t]]
    neg  = output[b, t, neg_ids[b, t, 0]]
    per_t = log_sigmoid(pos - neg)                # = -softplus(neg - pos)
    per_user = sum_t(per_t * (t < x_len[b])) / x_len[b]
    loss = -mean_b(per_user)

Only 2 of the 100000 items per (b, t) are touched, so instead of streaming
the 1.28 GB logits tensor we gather exactly the needed 800 scalars per core
with indirect (SWDGE) DMAs and do the tiny masked reduction on-chip.
Sharding: data-parallel over users, 4 users per core; each core emits its 4
per-user partials (sum_t softplus(neg-pos)*mask / x_len, positive) and the
host averages the 32 partials into the scalar loss.

HW indirect-DMA semantics (probed on this toolchain): each destination
PARTITION consumes one index (element units) from the offsets AP and
receives dest_free_size consecutive elements. Layout: t on partitions, one
(pos/neg, user) stream per column -> 8 gathers of [T=100 partitions, 1].
Index arithmetic stays < 2^24 (the DVE ALU is fp32 even for ints); the
per-user base u*T*N rides each DMA's element_offset (integer descriptor
math). The t*N and t tables are inline constants; raw engine streams with a
minimal semaphore chain avoid the Tile kernel-tail barrier butterfly.
"""

from contextlib import ExitStack

import numpy as np

B, T, N_ITEMS, S = 32, 100, 100000, 1
N_CORES = 8
BP = B // N_CORES      # users per core = 4
NC2 = 2 * BP           # pos|neg columns = 8

_CACHE = {}


def _build_nc(softplus=True):
    from concourse import bass, bacc, mybir

    f32 = mybir.dt.float32
    i32 = mybir.dt.int32

    nc = bacc.Bacc()
    xs = nc.declare_dram_parameter("xs", [BP * T, N_ITEMS], f32, isOutput=False)
    li = nc.declare_dram_parameter("li", [T, NC2], i32, isOutput=False)
    xl = nc.declare_dram_parameter("xl", [T, BP], f32, isOutput=False)
    res = nc.declare_dram_parameter("res", [1, BP], f32, isOutput=True)

    tn_d = nc.inline_tensor(
        np.broadcast_to(
            (np.arange(T, dtype=np.int64) * N_ITEMS)[:, None], (T, NC2)
        ).astype(np.int32),
        name="tn_const",
    )
    tio_d = nc.inline_tensor(
        np.broadcast_to(
            np.arange(T, dtype=np.float32)[:, None], (T, BP)
        ).copy(),
        name="tio_const",
    )

    with ExitStack() as stk:
        li_t = stk.enter_context(nc.sbuf_tensor([T, NC2], i32))
        tn_t = stk.enter_context(nc.sbuf_tensor([T, NC2], i32))
        gx = stk.enter_context(nc.sbuf_tensor([T, NC2], i32))
        vals = stk.enter_context(nc.sbuf_tensor([T, NC2], f32))
        xlf_t = stk.enter_context(nc.sbuf_tensor([T, BP], f32))
        tio_t = stk.enter_context(nc.sbuf_tensor([T, BP], f32))
        rcpf = stk.enter_context(nc.sbuf_tensor([T, BP], f32))
        msk = stk.enter_context(nc.sbuf_tensor([T, BP], f32))
        mskr = stk.enter_context(nc.sbuf_tensor([T, BP], f32))
        z = stk.enter_context(nc.sbuf_tensor([T, BP], f32))
        sp = stk.enter_context(nc.sbuf_tensor([T, BP], f32))
        spm = stk.enter_context(nc.sbuf_tensor([T, BP], f32))
        ones = stk.enter_context(nc.sbuf_tensor([T, 1], f32))
        sg_tmp = stk.enter_context(nc.sbuf_tensor([T, BP], f32))
        res_sb = stk.enter_context(nc.sbuf_tensor([1, BP], f32))
        acc = stk.enter_context(nc.psum_tensor("acc", [1, BP], f32))
        act_done = 2  # Exp, then Ln(bias=1)

        with (
            nc.Block() as block,
            nc.semaphore("s_dma") as s_dma,
            nc.semaphore("s_dge") as s_dge,
            nc.semaphore("s_v") as s_v,
            nc.semaphore("s_a") as s_a,
            nc.semaphore("s_p") as s_p,
        ):

            @block.sync
            def _(sync):
                sync.dma_start(out=li_t[:, :], in_=li[:, :]).then_inc(s_dma, 16)
                sync.dma_start(out=tn_t[:, :], in_=tn_d[:, :]).then_inc(s_dma, 16)
                sync.dma_start(out=xlf_t[:, :], in_=xl[:, :]).then_inc(s_dma, 16)
                sync.dma_start(out=tio_t[:, :], in_=tio_d[:, :]).then_inc(s_dma, 16)
                sync.wait_ge(s_v, 8)
                sync.dma_start(out=res[:, :], in_=res_sb[:, :]).then_inc(s_dma, 16)
                sync.wait_ge(s_dma, 80)

            @block.vector
            def _(vector):
                # DVE instructions pipeline: same-engine RAW needs the s_v
                # chain too. s_v counts every vector op below in order.
                # gx = li + t*N  (both < 2^24: exact in the fp32 DVE ALU).
                # One wait for all 4 input DMAs: HWDGE completions are
                # unordered, so partial counts can't identify which landed.
                vector.wait_ge(s_dma, 64)
                vector.tensor_add(
                    out=gx[:, :], in0=li_t[:, :], in1=tn_t[:, :]
                ).then_inc(s_v, 1)                                        # 1
                # mask/(x_len) pieces overlap the gathers
                vector.tensor_tensor(
                    out=msk[:, :], in0=tio_t[:, :], in1=xlf_t[:, :],
                    op=mybir.AluOpType.is_lt,
                ).then_inc(s_v, 1)                                        # 2
                vector.reciprocal(out=rcpf[:, :], in_=xlf_t[:, :]).then_inc(
                    s_v, 1
                )                                                         # 3
                vector.wait_ge(s_v, 3)
                vector.tensor_mul(
                    out=mskr[:, :], in0=msk[:, :], in1=rcpf[:, :]
                ).then_inc(s_v, 1)                                        # 4
                vector.memset(ones[:, :], 1.0).then_inc(s_v, 1)           # 5
                # z = neg - pos
                vector.wait_ge(s_dge, 16 * NC2)
                vector.tensor_sub(
                    out=z[:, :], in0=vals[:, BP:NC2], in1=vals[:, 0:BP]
                ).then_inc(s_v, 1)                                        # 6
                # spm = softplus(z) * mask / x_len
                vector.wait_ge(s_a, act_done)
                vector.wait_ge(s_v, 6)
                vector.tensor_mul(
                    out=spm[:, :], in0=sp[:, :], in1=mskr[:, :]
                ).then_inc(s_v, 1)                                        # 7
                # PSUM -> SBUF
                vector.wait_ge(s_p, 1)
                vector.tensor_copy(out=res_sb[:, :], in_=acc[:, :]).then_inc(
                    s_v, 1
                )                                                         # 8

            @block.gpsimd
            def _(gpsimd):
                gpsimd.wait_ge(s_v, 1)
                for c in range(NC2):
                    gpsimd.indirect_dma_start(
                        out=vals[:, c : c + 1],
                        out_offset=None,
                        in_=xs[:, :],
                        in_offset=bass.IndirectOffsetOnAxis(
                            ap=gx[:, c : c + 1], axis=1
                        ),
                        element_offset=(c % BP) * T * N_ITEMS,
                    ).then_inc(s_dge, 16)

            @block.scalar
            def _(scalar):
                # softplus(z) = Ln(Exp(z) + 1): Exp and Ln share one ACT
                # table (natural_log_exp_and_others), so no mid-kernel
                # table swap; the +1 rides Ln's bias port.
                scalar.wait_ge(s_v, 6)
                scalar.activation(
                    sg_tmp[:, :], z[:, :], mybir.ActivationFunctionType.Exp
                ).then_inc(s_a, 1)
                scalar.wait_ge(s_a, 1)
                scalar.activation(
                    sp[:, :], sg_tmp[:, :], mybir.ActivationFunctionType.Ln,
                    bias=1.0,
                ).then_inc(s_a, 1)

            @block.tensor
            def _(tensor):
                tensor.wait_ge(s_v, 7)
                tensor.matmul(
                    out=acc[:, :], lhsT=ones[:, :], rhs=spm[:, :],
                    start=True, stop=True,
                ).then_inc(s_p, 1)

    if not nc.is_finalized():
        nc.finalize()
    return nc, softplus


def _get_nc():
    if "nc" not in _CACHE:
        _CACHE["nc"] = _build_nc()
    return _CACHE["nc"]


def _make_in_maps(output, labels, x_lens, neg_ids):
    output = np.asarray(output, dtype=np.float32)
    labels = np.asarray(labels).astype(np.int32)
    neg = np.asarray(neg_ids).astype(np.int32).reshape(B, T * S)
    xlf = np.asarray(x_lens).astype(np.float32)

    in_maps = []
    for c in range(N_CORES):
        sl = slice(c * BP, (c + 1) * BP)
        li = np.concatenate([labels[sl].T, neg[sl].T], axis=1)  # [T, 2*BP]
        in_maps.append(
            {
                "xs": output[sl].reshape(BP * T, N_ITEMS),
                "li": np.ascontiguousarray(li),
                "xl": np.ascontiguousarray(
                    np.broadcast_to(xlf[sl][None, :], (T, BP))
                ),
            }
        )
    return in_maps


def run(output, labels, x_lens, neg_ids, uids=None, trace=False):
    """Run the SPMD bass kernel; returns (loss_scalar, BassKernelResults)."""
    from concourse.bass_utils import run_bass_kernel_spmd

    nc, softplus = _get_nc()
    in_maps = _make_in_maps(output, labels, x_lens, neg_ids)
    out = run_bass_kernel_spmd(nc, in_maps, list(range(N_CORES)), trace=trace)
    per_user = np.concatenate([r["res"].reshape(-1) for r in out.results])
    per_user = np.asarray(per_user, dtype=np.float32)
    # softplus partials are positive (= -log_sigmoid); Ln(Sigmoid) negative.
    loss = per_user.mean(dtype=np.float32)
    if not softplus:
        loss = -loss
    return np.float32(loss), out


def kernel(output, labels, x_lens, neg_ids, uids=None, **_ignored):
    loss, _ = run(output, labels, x_lens, neg_ids)
    return loss


# revision 23
# speedup vs baseline: 1.0808x; 1.0808x over previous
"""BPR loss kernel for Trainium2 (Bass, raw engine streams), SPMD over 8 cores.

Reference computation (B=32, T=100, N=100000, S=1):
    pos  = output[b, t, labels[b, t]]
    neg  = output[b, t, neg_ids[b, t, 0]]
    per_t = log_sigmoid(pos - neg)                # = -softplus(neg - pos)
    per_user = sum_t(per_t * (t < x_len[b])) / x_len[b]
    loss = -mean_b(per_user)

Only 2 of the 100000 items per (b, t) are touched, so instead of streaming
the 1.28 GB logits tensor we gather exactly the needed 800 scalars per core
with indirect (SWDGE) DMAs and do the tiny masked reduction on-chip.
Sharding: data-parallel over users, 4 users per core; each core emits its 4
per-user partials (sum_t softplus(neg-pos)*mask / x_len, positive) and the
host averages the 32 partials into the scalar loss.

HW indirect-DMA semantics (probed on this toolchain): each destination
PARTITION consumes one index (element units) from the offsets AP and
receives dest_free_size consecutive elements. Layout: t on partitions, one
(pos/neg, user) stream per column -> 8 gathers of [T=100 partitions, 1].
Index arithmetic stays < 2^24 (the DVE ALU is fp32 even for ints); the
per-user base u*T*N rides each DMA's element_offset (integer descriptor
math).

Perf structure: all 4 small operands ride ONE packed [T, 24]-word input DMA
(int32 with f32 columns bitcast on SBUF); softplus(z) = Ln(Exp(z) + 1) so
both ACT funcs share one table (natural_log_exp_and_others - enforced by
narrowing the table-picker's view during build; ids stay aligned with the
compiler's act_info.json); Block(no_gpsimd_drain=True) exits via the
sem-only barrier instead of the EVSEM butterfly + SWDGE dge-drain.
"""

from contextlib import ExitStack

import numpy as np

B, T, N_ITEMS, S = 32, 100, 100000, 1
N_CORES = 8
BP = B // N_CORES      # users per core = 4
NC2 = 2 * BP           # pos|neg columns = 8
PKW = 2 * NC2 + 2 * BP  # packed input words per row: li(8) tn(8) xl(4) tio(4)

_CACHE = {}


def _build_nc():
    from concourse import bass, bacc, mybir

    f32 = mybir.dt.float32
    i32 = mybir.dt.int32

    nc = bacc.Bacc()
    xs = nc.declare_dram_parameter("xs", [BP * T, N_ITEMS], f32, isOutput=False)
    pk = nc.declare_dram_parameter("pk", [T, PKW], i32, isOutput=False)
    res = nc.declare_dram_parameter("res", [1, BP], f32, isOutput=True)

    with ExitStack() as stk:
        pk_t = stk.enter_context(nc.sbuf_tensor([T, PKW], i32))
        gx = stk.enter_context(nc.sbuf_tensor([T, NC2], i32))
        vals = stk.enter_context(nc.sbuf_tensor([T, NC2], f32))
        rcpf = stk.enter_context(nc.sbuf_tensor([T, BP], f32))
        msk = stk.enter_context(nc.sbuf_tensor([T, BP], f32))
        mskr = stk.enter_context(nc.sbuf_tensor([T, BP], f32))
        z = stk.enter_context(nc.sbuf_tensor([T, BP], f32))
        ez = stk.enter_context(nc.sbuf_tensor([T, BP], f32))
        sp = stk.enter_context(nc.sbuf_tensor([T, BP], f32))
        spm = stk.enter_context(nc.sbuf_tensor([T, BP], f32))
        ones = stk.enter_context(nc.sbuf_tensor([T, 1], f32))
        res_sb = stk.enter_context(nc.sbuf_tensor([1, BP], f32))
        acc = stk.enter_context(nc.psum_tensor("acc", [1, BP], f32))

        li_ap = pk_t[:, 0:NC2]
        tn_ap = pk_t[:, NC2 : 2 * NC2]
        xlf_ap = pk_t[:, 2 * NC2 : 2 * NC2 + BP].bitcast(f32)
        tio_ap = pk_t[:, 2 * NC2 + BP : PKW].bitcast(f32)

        with (
            nc.Block(no_gpsimd_drain=True) as block,
            nc.semaphore("s_dma") as s_dma,
            nc.semaphore("s_dge") as s_dge,
            nc.semaphore("s_v") as s_v,
            nc.semaphore("s_a") as s_a,
            nc.semaphore("s_p") as s_p,
        ):

            @block.sync
            def _(sync):
                sync.dma_start(out=pk_t[:, :], in_=pk[:, :]).then_inc(s_dma, 16)
                sync.wait_ge(s_v, 8)
                sync.dma_start(out=res[:, :], in_=res_sb[:, :]).then_inc(s_dma, 16)
                sync.wait_ge(s_dma, 32)

            @block.vector
            def _(vector):
                # DVE instructions pipeline: same-engine RAW needs the s_v
                # chain too. s_v counts every producing vector op in order.
                # gx = li + t*N (both < 2^24: exact in the fp32 DVE ALU).
                vector.wait_ge(s_dma, 16)
                vector.tensor_add(
                    out=gx[:, :], in0=li_ap, in1=tn_ap
                ).then_inc(s_v, 1)                                        # 1
                # mask/(x_len) pieces overlap the gathers
                vector.tensor_tensor(
                    out=msk[:, :], in0=tio_ap, in1=xlf_ap,
                    op=mybir.AluOpType.is_lt,
                ).then_inc(s_v, 1)                                        # 2
                vector.reciprocal(out=rcpf[:, :], in_=xlf_ap).then_inc(
                    s_v, 1
                )                                                         # 3
                vector.wait_ge(s_v, 3)
                vector.tensor_mul(
                    out=mskr[:, :], in0=msk[:, :], in1=rcpf[:, :]
                ).then_inc(s_v, 1)                                        # 4
                vector.memset(ones[:, :], 1.0).then_inc(s_v, 1)           # 5
                # z = neg - pos
                vector.wait_ge(s_dge, 16 * NC2)
                vector.tensor_sub(
                    out=z[:, :], in0=vals[:, BP:NC2], in1=vals[:, 0:BP]
                ).then_inc(s_v, 1)                                        # 6
                # spm = softplus(z) * mask / x_len
                vector.wait_ge(s_a, 2)
                vector.wait_ge(s_v, 6)
                vector.tensor_mul(
                    out=spm[:, :], in0=sp[:, :], in1=mskr[:, :]
                ).then_inc(s_v, 1)                                        # 7
                # PSUM -> SBUF
                vector.wait_ge(s_p, 1)
                vector.tensor_copy(out=res_sb[:, :], in_=acc[:, :]).then_inc(
                    s_v, 1
                )                                                         # 8

            @block.gpsimd
            def _(gpsimd):
                gpsimd.wait_ge(s_v, 1)
                for c in range(NC2):
                    gpsimd.indirect_dma_start(
                        out=vals[:, c : c + 1],
                        out_offset=None,
                        in_=xs[:, :],
                        in_offset=bass.IndirectOffsetOnAxis(
                            ap=gx[:, c : c + 1], axis=1
                        ),
                        element_offset=(c % BP) * T * N_ITEMS,
                    ).then_inc(s_dge, 16)

            @block.scalar
            def _(scalar):
                # softplus(z) = Ln(Exp(z) + 1); Exp and Ln share one ACT
                # table, so the single table load overlaps the gathers.
                scalar.wait_ge(s_v, 6)
                scalar.activation(
                    ez[:, :], z[:, :], mybir.ActivationFunctionType.Exp
                ).then_inc(s_a, 1)
                scalar.wait_ge(s_a, 1)
                scalar.activation(
                    sp[:, :], ez[:, :], mybir.ActivationFunctionType.Ln,
                    bias=1.0,
                ).then_inc(s_a, 1)

            @block.tensor
            def _(tensor):
                tensor.wait_ge(s_v, 7)
                tensor.matmul(
                    out=acc[:, :], lhsT=ones[:, :], rhs=spm[:, :],
                    start=True, stop=True,
                ).then_inc(s_p, 1)

    _finalize_with_shared_act_table(nc)
    return nc


def _finalize_with_shared_act_table(nc):
    """Finalize with the ACT table-picker constrained so Exp and Ln both
    resolve to natural_log_exp_and_others (one load, no mid-kernel table
    swap). Table ids/order are untouched, so InstLoadActFuncSet ids still
    match the compiler's act_info.json. Patch is restored afterwards."""
    from concourse import bacc, hw_specs, mybir

    target = "natural_log_exp_and_others"
    orig = hw_specs.get_activation_tables

    def narrowed(arch):
        tabs = orig(arch)
        if target in tabs:
            for name, fns in tabs.items():
                if name != target:
                    fns.discard(mybir.ActivationFunctionType.Exp)
                    fns.discard(mybir.ActivationFunctionType.Ln)
        return tabs

    hw_specs.get_activation_tables = narrowed
    bacc.get_activation_tables = narrowed
    try:
        if not nc.is_finalized():
            nc.finalize()
    finally:
        hw_specs.get_activation_tables = orig
        bacc.get_activation_tables = orig


def _get_nc():
    if "nc" not in _CACHE:
        _CACHE["nc"] = _build_nc()
    return _CACHE["nc"]


def _make_in_maps(output, labels, x_lens, neg_ids):
    output = np.asarray(output, dtype=np.float32)
    labels = np.asarray(labels).astype(np.int32)
    neg = np.asarray(neg_ids).astype(np.int32).reshape(B, T * S)
    xlf = np.asarray(x_lens).astype(np.float32)

    tn = np.broadcast_to(
        (np.arange(T, dtype=np.int64) * N_ITEMS)[:, None], (T, NC2)
    ).astype(np.int32)
    tio = np.broadcast_to(np.arange(T, dtype=np.float32)[:, None], (T, BP))

    in_maps = []
    for c in range(N_CORES):
        sl = slice(c * BP, (c + 1) * BP)
        li = np.concatenate([labels[sl].T, neg[sl].T], axis=1)  # [T, 2*BP]
        xl_rep = np.broadcast_to(xlf[sl][None, :], (T, BP))
        pk = np.concatenate(
            [
                li.astype(np.int32),
                tn,
                xl_rep.astype(np.float32).view(np.int32),
                tio.astype(np.float32).view(np.int32),
            ],
            axis=1,
        )
        in_maps.append(
            {
                "xs": output[sl].reshape(BP * T, N_ITEMS),
                "pk": np.ascontiguousarray(pk),
            }
        )
    return in_maps


def run(output, labels, x_lens, neg_ids, uids=None, trace=False):
    """Run the SPMD bass kernel; returns (loss_scalar, BassKernelResults)."""
    from concourse.bass_utils import run_bass_kernel_spmd

    nc = _get_nc()
    in_maps = _make_in_maps(output, labels, x_lens, neg_ids)
    out = run_bass_kernel_spmd(nc, in_maps, list(range(N_CORES)), trace=trace)
    # res holds positive per-user partials (softplus = -log_sigmoid).
    per_user = np.concatenate([r["res"].reshape(-1) for r in out.results])
    loss = np.asarray(per_user, dtype=np.float32).mean(dtype=np.float32)
    return np.float32(loss), out


def kernel(output, labels, x_lens, neg_ids, uids=None, **_ignored):
    loss, _ = run(output, labels, x_lens, neg_ids)
    return loss


# revision 29
# speedup vs baseline: 1.1135x; 1.0303x over previous
"""BPR loss kernel for Trainium2 (Bass, raw engine streams), SPMD over 8 cores.

Reference computation (B=32, T=100, N=100000, S=1):
    pos  = output[b, t, labels[b, t]]
    neg  = output[b, t, neg_ids[b, t, 0]]
    per_t = log_sigmoid(pos - neg)                # = -softplus(neg - pos)
    per_user = sum_t(per_t * (t < x_len[b])) / x_len[b]
    loss = -mean_b(per_user)

Only 2 of the 100000 items per (b, t) are touched, so instead of streaming
the 1.28 GB logits tensor we gather exactly the needed 800 scalars per core
with indirect (SWDGE) DMAs and do the tiny masked reduction on-chip.
Sharding: data-parallel over users, 4 users per core; each core emits its 4
per-user partials (sum_t softplus(neg-pos)*mask / x_len, positive) and the
host averages the 32 partials into the scalar loss.

HW indirect-DMA semantics (probed on this toolchain): each destination
PARTITION consumes one index (element units) from the offsets AP and
receives dest_free_size consecutive elements. Layout: t on partitions, one
(pos/neg, user) stream per column -> 8 gathers of [T=100 partitions, 1].
Index arithmetic stays < 2^24 (the DVE ALU is fp32 even for ints); the
per-user base u*T*N rides each DMA's element_offset (integer descriptor
math).

Perf structure: all 4 small operands ride ONE packed [T, 24]-word input DMA
(int32 with f32 columns bitcast on SBUF); softplus(z) = Ln(Exp(z) + 1) so
both ACT funcs share one table (natural_log_exp_and_others - enforced by
narrowing the table-picker's view during build; ids stay aligned with the
compiler's act_info.json); Block(no_gpsimd_drain=True) exits via the
sem-only barrier instead of the EVSEM butterfly + SWDGE dge-drain.
"""

from contextlib import ExitStack

import numpy as np

B, T, N_ITEMS, S = 32, 100, 100000, 1
N_CORES = 8
BP = B // N_CORES      # users per core = 4
NC2 = 2 * BP           # pos|neg columns = 8
PKW = 2 * NC2 + 2 * BP  # packed input words per row: li(8) tn(8) xl(4) tio(4)

_CACHE = {}


def _build_nc():
    from concourse import bass, bacc, mybir

    f32 = mybir.dt.float32
    i32 = mybir.dt.int32

    nc = bacc.Bacc()
    xs = nc.declare_dram_parameter("xs", [BP * T, N_ITEMS], f32, isOutput=False)
    pk = nc.declare_dram_parameter("pk", [T, PKW], i32, isOutput=False)
    res = nc.declare_dram_parameter("res", [1, BP], f32, isOutput=True)

    with ExitStack() as stk:
        pk_t = stk.enter_context(nc.sbuf_tensor([T, PKW], i32))
        gx = stk.enter_context(nc.sbuf_tensor([T, NC2], i32))
        vals = stk.enter_context(nc.sbuf_tensor([T, NC2], f32))
        rcpf = stk.enter_context(nc.sbuf_tensor([T, BP], f32))
        msk = stk.enter_context(nc.sbuf_tensor([T, BP], f32))
        mskr = stk.enter_context(nc.sbuf_tensor([T, BP], f32))
        z = stk.enter_context(nc.sbuf_tensor([T, BP], f32))
        ez = stk.enter_context(nc.sbuf_tensor([T, BP], f32))
        sp = stk.enter_context(nc.sbuf_tensor([T, BP], f32))
        spm = stk.enter_context(nc.sbuf_tensor([T, BP], f32))
        ones = stk.enter_context(nc.sbuf_tensor([T, 1], f32))
        res_sb = stk.enter_context(nc.sbuf_tensor([1, BP], f32))
        acc = stk.enter_context(nc.psum_tensor("acc", [1, BP], f32))

        li_ap = pk_t[:, 0:NC2]
        tn_ap = pk_t[:, NC2 : 2 * NC2]
        xlf_ap = pk_t[:, 2 * NC2 : 2 * NC2 + BP].bitcast(f32)
        tio_ap = pk_t[:, 2 * NC2 + BP : PKW].bitcast(f32)

        with (
            nc.Block(no_gpsimd_drain=True) as block,
            nc.semaphore("s_dma") as s_dma,
            nc.semaphore("s_dge") as s_dge,
            nc.semaphore("s_v") as s_v,
            nc.semaphore("s_a") as s_a,
            nc.semaphore("s_p") as s_p,
            nc.semaphore("s_g2") as s_g2,
        ):

            @block.sync
            def _(sync):
                sync.dma_start(out=pk_t[:, :], in_=pk[:, :]).then_inc(s_dma, 16)
                sync.wait_ge(s_v, 7)
                sync.dma_start(out=res[:, :], in_=res_sb[:, :]).then_inc(s_dma, 16)
                sync.wait_ge(s_dma, 32)

            @block.vector
            def _(vector):
                # DVE instructions pipeline: same-engine RAW needs the s_v
                # chain too. s_v counts every producing vector op in order.
                # mask/(x_len) pieces overlap the gathers
                vector.wait_ge(s_dma, 16)
                vector.tensor_tensor(
                    out=msk[:, :], in0=tio_ap, in1=xlf_ap,
                    op=mybir.AluOpType.is_lt,
                ).then_inc(s_v, 1)                                        # 1
                vector.reciprocal(out=rcpf[:, :], in_=xlf_ap).then_inc(
                    s_v, 1
                )                                                         # 2
                vector.wait_ge(s_v, 2)
                vector.tensor_mul(
                    out=mskr[:, :], in0=msk[:, :], in1=rcpf[:, :]
                ).then_inc(s_v, 1)                                        # 3
                vector.memset(ones[:, :], 1.0).then_inc(s_v, 1)           # 4
                # z = neg - pos
                vector.wait_ge(s_dge, 16 * NC2)
                vector.tensor_sub(
                    out=z[:, :], in0=vals[:, BP:NC2], in1=vals[:, 0:BP]
                ).then_inc(s_v, 1)                                        # 5
                # spm = softplus(z) * mask / x_len
                vector.wait_ge(s_a, 2)
                vector.wait_ge(s_v, 5)
                vector.tensor_mul(
                    out=spm[:, :], in0=sp[:, :], in1=mskr[:, :]
                ).then_inc(s_v, 1)                                        # 6
                # PSUM -> SBUF
                vector.wait_ge(s_p, 1)
                vector.tensor_copy(out=res_sb[:, :], in_=acc[:, :]).then_inc(
                    s_v, 1
                )                                                         # 7

            @block.gpsimd
            def _(gpsimd):
                # gx = li + t*N on the Q7 (exact int32 ALU, no cross-engine
                # hop to the gathers that follow on this same engine).
                gpsimd.wait_ge(s_dma, 16)
                gpsimd.tensor_tensor(
                    out=gx[:, :], in0=li_ap, in1=tn_ap, op=mybir.AluOpType.add
                ).then_inc(s_g2, 1)
                gpsimd.wait_ge(s_g2, 1)
                # walrus codegen requires a sem update on every DMACopy.
                for c in range(NC2):
                    gpsimd.indirect_dma_start(
                        out=vals[:, c : c + 1],
                        out_offset=None,
                        in_=xs[:, :],
                        in_offset=bass.IndirectOffsetOnAxis(
                            ap=gx[:, c : c + 1], axis=1
                        ),
                        element_offset=(c % BP) * T * N_ITEMS,
                    ).then_inc(s_dge, 16)

            @block.scalar
            def _(scalar):
                # softplus(z) = Ln(Exp(z) + 1); Exp and Ln share one ACT
                # table, so the single table load overlaps the gathers.
                scalar.wait_ge(s_v, 5)
                scalar.activation(
                    ez[:, :], z[:, :], mybir.ActivationFunctionType.Exp
                ).then_inc(s_a, 1)
                scalar.wait_ge(s_a, 1)
                scalar.activation(
                    sp[:, :], ez[:, :], mybir.ActivationFunctionType.Ln,
                    bias=1.0,
                ).then_inc(s_a, 1)

            @block.tensor
            def _(tensor):
                tensor.wait_ge(s_v, 6)
                tensor.matmul(
                    out=acc[:, :], lhsT=ones[:, :], rhs=spm[:, :],
                    start=True, stop=True,
                ).then_inc(s_p, 1)

    _finalize_with_shared_act_table(nc)
    return nc


def _finalize_with_shared_act_table(nc):
    """Finalize with the ACT table-picker constrained so Exp and Ln both
    resolve to natural_log_exp_and_others (one load, no mid-kernel table
    swap). Table ids/order are untouched, so InstLoadActFuncSet ids still
    match the compiler's act_info.json. Patch is restored afterwards."""
    from concourse import bacc, hw_specs, mybir

    target = "natural_log_exp_and_others"
    orig = hw_specs.get_activation_tables

    def narrowed(arch):
        tabs = orig(arch)
        if target in tabs:
            for name, fns in tabs.items():
                if name != target:
                    fns.discard(mybir.ActivationFunctionType.Exp)
                    fns.discard(mybir.ActivationFunctionType.Ln)
        return tabs

    hw_specs.get_activation_tables = narrowed
    bacc.get_activation_tables = narrowed
    try:
        if not nc.is_finalized():
            nc.finalize()
    finally:
        hw_specs.get_activation_tables = orig
        bacc.get_activation_tables = orig


def _get_nc():
    if "nc" not in _CACHE:
        _CACHE["nc"] = _build_nc()
    return _CACHE["nc"]


def _make_in_maps(output, labels, x_lens, neg_ids):
    output = np.asarray(output, dtype=np.float32)
    labels = np.asarray(labels).astype(np.int32)
    neg = np.asarray(neg_ids).astype(np.int32).reshape(B, T * S)
    xlf = np.asarray(x_lens).astype(np.float32)

    tn = np.broadcast_to(
        (np.arange(T, dtype=np.int64) * N_ITEMS)[:, None], (T, NC2)
    ).astype(np.int32)
    tio = np.broadcast_to(np.arange(T, dtype=np.float32)[:, None], (T, BP))

    in_maps = []
    for c in range(N_CORES):
        sl = slice(c * BP, (c + 1) * BP)
        li = np.concatenate([labels[sl].T, neg[sl].T], axis=1)  # [T, 2*BP]
        xl_rep = np.broadcast_to(xlf[sl][None, :], (T, BP))
        pk = np.concatenate(
            [
                li.astype(np.int32),
                tn,
                xl_rep.astype(np.float32).view(np.int32),
                tio.astype(np.float32).view(np.int32),
            ],
            axis=1,
        )
        in_maps.append(
            {
                "xs": output[sl].reshape(BP * T, N_ITEMS),
                "pk": np.ascontiguousarray(pk),
            }
        )
    return in_maps


def run(output, labels, x_lens, neg_ids, uids=None, trace=False):
    """Run the SPMD bass kernel; returns (loss_scalar, BassKernelResults)."""
    from concourse.bass_utils import run_bass_kernel_spmd

    nc = _get_nc()
    in_maps = _make_in_maps(output, labels, x_lens, neg_ids)
    out = run_bass_kernel_spmd(nc, in_maps, list(range(N_CORES)), trace=trace)
    # res holds positive per-user partials (softplus = -log_sigmoid).
    per_user = np.concatenate([r["res"].reshape(-1) for r in out.results])
    loss = np.asarray(per_user, dtype=np.float32).mean(dtype=np.float32)
    return np.float32(loss), out


def kernel(output, labels, x_lens, neg_ids, uids=None, **_ignored):
    loss, _ = run(output, labels, x_lens, neg_ids)
    return loss
